# revision 14
# baseline (speedup 1.0000x reference)
"""Single-head full-attention layer on 8 Trainium2 NeuronCores (fp8 DoubleRow).

reference:
    q = seq @ Wq; k = seq @ Wk; v = seq @ Wv          # [B,S,D], D=1024
    scores = q @ k.T / sqrt(D)                        # [B,S,S]
    out = seq + softmax(scores) @ v * mask            # [B,S,D]

Sharding: 8 cores = 4 batches x 2 sequence-halves. Each core computes Q
for its own 1024 queries and K^T/V for its own 1024 keys; pairs exchange
K^T/V halves via 2-core AllGathers; each core then runs
softmax(QK^T)V + mask + residual for its query half.

All matmuls run in fp8(E4M3) with perf_mode=DoubleRow (two contraction
rows per PE cell): operands live in 3D SBUF tiles [128, ksub, free] and
each matmul consumes a [:, k:k+2, :] slice. Numerics:
  - host scales W by 32 before the fp8 cast (keeps N(0,1/1024) weights
    out of the fp8 subnormal range); the q/k factors cancel inside
    exp's scale (2^-15) and V's factor is folded into 1/colsum.
  - exp is shifted by -3 so attn values stay below fp8e4's +-240 max
    (softmax is shift-invariant).
  - scores/colsum/outT accumulate in fp32 PSUM; the normalization,
    output mask (folded into Wv on host) and fp32 residual are applied
    in the final fp32 stage.

The SPMD program is identical on all cores; the AllGather delivers keys
in global order for everyone.
"""

import numpy as np
import ml_dtypes

import concourse.bass as bass
import concourse.mybir as mybir
import concourse.tile as tile
from concourse import bacc, bass_utils

B, S, D = 4, 2048, 1024
N_CORES = 8
SH = S // 2          # queries / own keys per core
PD = 128             # partition dim
KD = D // PD         # 8 ksub chunks over d
KH = SH // PD        # 8 ksub chunks over own keys
KC = S // PD         # 16 ksub chunks over all keys
NT = 512             # matmul free-dim tile (one PSUM bank of fp32)
F8 = mybir.dt.float8e4
F32 = mybir.dt.float32
W_SCALE = 32.0
EXP_SCALE = 1.0 / (32.0 * W_SCALE * W_SCALE)   # 1/sqrt(D) / W_SCALE^2
EXP_SHIFT = -3.0
DR = mybir.MatmulPerfMode.DoubleRow

_FP8 = ml_dtypes.float8_e4m3
_GROUPS = [[0, 1], [2, 3], [4, 5], [6, 7]]


def _build_kernel(tc):
    nc = tc.nc
    seqTq = nc.dram_tensor("seqTq", [D, SH], F8, kind="ExternalInput").ap()
    wq = nc.dram_tensor("wq", [D, D], F8, kind="ExternalInput").ap()
    wk = nc.dram_tensor("wk", [D, D], F8, kind="ExternalInput").ap()
    wv = nc.dram_tensor("wv", [D, D], F8, kind="ExternalInput").ap()
    seqTh = nc.dram_tensor("seqTh", [SH, D], F32, kind="ExternalInput").ap()
    outT = nc.dram_tensor("outT", [SH, D], F32, kind="ExternalOutput").ap()

    Exp = mybir.ActivationFunctionType.Exp

    with (
        tc.tile_pool(name="p_seq", bufs=1) as p_seq,
        tc.tile_pool(name="p_w", bufs=2) as p_w,
        tc.tile_pool(name="p_own", bufs=1) as p_own,
        tc.tile_pool(name="p_qt", bufs=1) as p_qt,
        tc.tile_pool(name="p_kt", bufs=1) as p_kt,
        tc.tile_pool(name="p_v", bufs=1) as p_v,
        tc.tile_pool(name="p_at", bufs=1) as p_at,
        tc.tile_pool(name="p_sh", bufs=4) as p_sh,
        tc.tile_pool(name="p_o", bufs=4) as p_o,
        tc.tile_pool(name="p_msc", bufs=1) as p_msc,
        tc.tile_pool(name="p_dram", bufs=1, space="DRAM") as p_dram,
        tc.tile_pool(name="p_mm", bufs=6, space="PSUM") as p_mm,
        tc.tile_pool(name="p_cs", bufs=1, space="PSUM") as p_cs,
    ):
        # ---- resident inputs ------------------------------------------------
        def load_w(w_dram, label):
            t = p_w.tile([PD, KD, D], F8, tag="w", name=label)
            for j in range(KD):
                nc.sync.dma_start(t[:, j, :], w_dram[j * PD:(j + 1) * PD, :])
            return t

        # Wk first (first matmul chain needs it), then own-half seq columns.
        wk3 = load_w(wk, "wk3")
        seqq3 = p_seq.tile([PD, KD, SH], F8, tag="seqq", name="seqq3")
        for j in range(KD):
            nc.sync.dma_start(seqq3[:, j, :], seqTq[j * PD:(j + 1) * PD, :])

        # collective bounce buffers (DRAM, Local)
        ib_kt = p_dram.tile([D, SH], F8, tag="ibk", name="ib_kt")
        ob_kt = p_dram.tile([2, D, SH], F8, tag="obk", name="ob_kt")
        ib_v = p_dram.tile([SH, D], F8, tag="ibv", name="ib_v")
        ob_v = p_dram.tile([2, SH, D], F8, tag="obv", name="ob_v")

        # ---- KT_own = (seq_own @ Wk).T, bounce out, AllGather ---------------
        kto3 = p_own.tile([PD, KD, SH], F8, tag="kto", name="kto3")
        for m in range(KD):
            for n in range(SH // NT):
                ps = p_mm.tile([PD, NT], F32, tag="mm", name=f"ps_k{m}_{n}")
                for k in range(0, KD, 2):
                    nc.tensor.matmul(
                        ps[:],
                        wk3[:, k:k + 2, m * PD:(m + 1) * PD],
                        seqq3[:, k:k + 2, n * NT:(n + 1) * NT],
                        start=(k == 0),
                        stop=(k == KD - 2),
                        perf_mode=DR,
                    )
                nc.vector.tensor_copy(kto3[:, m, n * NT:(n + 1) * NT], ps[:])
            nc.sync.dma_start(ib_kt[m * PD:(m + 1) * PD, :], kto3[:, m, :])
        nc.gpsimd.collective_compute(
            "AllGather", mybir.AluOpType.bypass, replica_groups=_GROUPS,
            ins=[ib_kt.opt()], outs=[ob_kt.opt()],
        )

        # ---- V_own = seq_own @ (Wv * mask), bounce out, AllGather -----------
        wv3 = load_w(wv, "wv3")
        vo3 = p_own.tile([PD, KH, D], F8, tag="vo", name="vo3")
        for m in range(KH):
            for n in range(D // NT):
                ps = p_mm.tile([PD, NT], F32, tag="mm", name=f"ps_v{m}_{n}")
                for k in range(0, KD, 2):
                    nc.tensor.matmul(
                        ps[:],
                        seqq3[:, k:k + 2, m * PD:(m + 1) * PD],
                        wv3[:, k:k + 2, n * NT:(n + 1) * NT],
                        start=(k == 0),
                        stop=(k == KD - 2),
                        perf_mode=DR,
                    )
                nc.vector.tensor_copy(vo3[:, m, n * NT:(n + 1) * NT], ps[:])
            nc.sync.dma_start(ib_v[m * PD:(m + 1) * PD, :], vo3[:, m, :])
        nc.gpsimd.collective_compute(
            "AllGather", mybir.AluOpType.bypass, replica_groups=_GROUPS,
            ins=[ib_v.opt()], outs=[ob_v.opt()],
        )

        # ---- QT = (seq_own @ Wq).T (overlaps the collectives) ---------------
        wq3 = load_w(wq, "wq3")
        qt3 = p_qt.tile([PD, KD, SH], F8, tag="qt", name="qt3")
        for m in range(KD):
            for n in range(SH // NT):
                ps = p_mm.tile([PD, NT], F32, tag="mm", name=f"ps_q{m}_{n}")
                for k in range(0, KD, 2):
                    nc.tensor.matmul(
                        ps[:],
                        wq3[:, k:k + 2, m * PD:(m + 1) * PD],
                        seqq3[:, k:k + 2, n * NT:(n + 1) * NT],
                        start=(k == 0),
                        stop=(k == KD - 2),
                        perf_mode=DR,
                    )
                nc.vector.tensor_copy(qt3[:, m, n * NT:(n + 1) * NT], ps[:])

        # ---- gather exchanged KT / V into SBUF ------------------------------
        kt3 = p_kt.tile([PD, KD, S], F8, tag="kt", name="kt3")
        for m in range(KD):
            for r in range(2):
                nc.sync.dma_start(
                    kt3[:, m, r * SH:(r + 1) * SH],
                    ob_kt[r, m * PD:(m + 1) * PD, :],
                )
        v3t = p_v.tile([PD, KC, D], F8, tag="v", name="v3t")
        for m in range(KC):
            r, mm_ = divmod(m, KH)
            nc.sync.dma_start(v3t[:, m, :], ob_v[r, mm_ * PD:(mm_ + 1) * PD, :])

        # ---- scoresT -> exp(shifted) -> colsum ------------------------------
        ones3 = p_msc.tile([PD, 2, 16], F8, tag="ones", name="ones3")
        nc.vector.memset(ones3[:], 1.0)
        ebias = p_msc.tile([PD, 1], F32, tag="ebias", name="ebias")
        nc.vector.memset(ebias[:], EXP_SHIFT)
        cs_ps = p_cs.tile([1, SH], F32, tag="cs", name="cs")
        at3 = p_at.tile([PD, KC, SH], F8, tag="at", name="at3")

        def colsum_mm(m):
            for n in range(SH // NT):
                nc.tensor.matmul(
                    cs_ps[:, n * NT:(n + 1) * NT],
                    ones3[:, 0:2, 0:1],
                    at3[:, m:m + 2, n * NT:(n + 1) * NT],
                    start=(m == 0),
                    stop=(m == KC - 2),
                    perf_mode=DR,
                )

        for m in range(KC):
            for n in range(SH // NT):
                ps = p_mm.tile([PD, NT], F32, tag="mm", name=f"ps_s{m}_{n}")
                for k in range(0, KD, 2):
                    nc.tensor.matmul(
                        ps[:],
                        kt3[:, k:k + 2, m * PD:(m + 1) * PD],
                        qt3[:, k:k + 2, n * NT:(n + 1) * NT],
                        start=(k == 0),
                        stop=(k == KD - 2),
                        perf_mode=DR,
                    )
                nc.scalar.activation(
                    at3[:, m, n * NT:(n + 1) * NT], ps[:], Exp,
                    bias=ebias[:], scale=EXP_SCALE,
                )
            # colsum pairs ksubs (m, m+1); emit one pair late so the PE
            # never waits on ACT's exp
            if m >= 3 and m % 2 == 1:
                colsum_mm(m - 3)
        colsum_mm(KC - 2)

        # ---- 1/colsum (incl. V's W_SCALE), transposed to per-partition ------
        recip_sb = p_msc.tile([1, SH], F32, tag="recip", name="recip")
        nc.vector.reciprocal(recip_sb[:], cs_ps[:])
        nc.vector.tensor_scalar_mul(recip_sb[:], recip_sb[:], 1.0 / W_SCALE)
        recipT = p_msc.tile([PD, KH], F32, tag="recipT", name="recipT")
        for m in range(KH):
            nc.gpsimd.dma_start(recipT[:, m:m + 1], recip_sb[0:1, m * PD:(m + 1) * PD])

        # ---- O = AT.T @ V in [q, d] layout; fused normalize + residual ------
        # out[q, d] = (sum_key at[key, q] * v[key, d]) * recip[q] + seq[q, d]
        for m in range(KH):
            sh_t = p_sh.tile([PD, D], F32, tag="sh", name=f"sh{m}")
            nc.sync.dma_start(sh_t[:], seqTh[m * PD:(m + 1) * PD, :])
            o_t = p_o.tile([PD, D], F32, tag="o", name=f"o{m}")
            for n in range(D // NT):
                ps = p_mm.tile([PD, NT], F32, tag="mm", name=f"ps_o{m}_{n}")
                for k in range(0, KC, 2):
                    nc.tensor.matmul(
                        ps[:],
                        at3[:, k:k + 2, m * PD:(m + 1) * PD],
                        v3t[:, k:k + 2, n * NT:(n + 1) * NT],
                        start=(k == 0),
                        stop=(k == KC - 2),
                        perf_mode=DR,
                    )
                nc.vector.scalar_tensor_tensor(
                    o_t[:, n * NT:(n + 1) * NT],
                    ps[:],
                    recipT[:, m:m + 1],
                    sh_t[:, n * NT:(n + 1) * NT],
                    op0=mybir.AluOpType.mult,
                    op1=mybir.AluOpType.add,
                )
            nc.sync.dma_start(outT[m * PD:(m + 1) * PD, :], o_t[:])


_NC_CACHE = None


def _get_nc():
    global _NC_CACHE
    if _NC_CACHE is None:
        nc = bacc.Bacc(
            "TRN2", target_bir_lowering=False, debug=False, num_devices=N_CORES
        )
        with tile.TileContext(nc) as tc:
            _build_kernel(tc)
        nc.compile()
        _NC_CACHE = nc
    return _NC_CACHE


def _prep_in_maps(seq, Wq, Wk, Wv, mask):
    seq = np.asarray(seq, dtype=np.float32)
    wq_f8 = (np.asarray(Wq, dtype=np.float32) * W_SCALE).astype(_FP8)
    wk_f8 = (np.asarray(Wk, dtype=np.float32) * W_SCALE).astype(_FP8)
    wvm_f8 = (np.asarray(Wv, dtype=np.float32)
              * np.asarray(mask, dtype=np.float32)[None, :] * W_SCALE).astype(_FP8)
    in_maps = []
    for c in range(N_CORES):
        b, h = divmod(c, 2)
        seqT_own = np.ascontiguousarray(seq[b, h * SH:(h + 1) * SH, :].T)  # [D, SH]
        in_maps.append({
            "seqTq": seqT_own.astype(_FP8),
            "wq": wq_f8,
            "wk": wk_f8,
            "wv": wvm_f8,
            "seqTh": np.ascontiguousarray(seq[b, h * SH:(h + 1) * SH, :]),
        })
    return in_maps


def _run(seq, Wq, Wk, Wv, mask, trace=False, **run_kwargs):
    nc = _get_nc()
    in_maps = _prep_in_maps(seq, Wq, Wk, Wv, mask)
    res = bass_utils.run_bass_kernel_spmd(
        nc, in_maps, core_ids=list(range(N_CORES)), trace=trace, **run_kwargs
    )
    out = np.empty((B, S, D), dtype=np.float32)
    for c in range(N_CORES):
        b, h = divmod(c, 2)
        out[b, h * SH:(h + 1) * SH, :] = res.results[c]["outT"]
    return out, res


def kernel(seq, Wq, Wk, Wv, mask):
    out, _ = _run(seq, Wq, Wk, Wv, mask)
    return out


# revision 16
# speedup vs baseline: 1.0090x; 1.0090x over previous
"""Single-head full-attention layer on 8 Trainium2 NeuronCores (fp8 DoubleRow).

reference:
    q = seq @ Wq; k = seq @ Wk; v = seq @ Wv          # [B,S,D], D=1024
    scores = q @ k.T / sqrt(D)                        # [B,S,S]
    out = seq + softmax(scores) @ v * mask            # [B,S,D]

Sharding: 8 cores = 4 batches x 2 sequence-halves. Each core computes Q
for its own 1024 queries and K^T/V for its own 1024 keys; pairs exchange
K^T/V halves via 2-core AllGathers; each core then runs
softmax(QK^T)V + mask + residual for its query half.

All matmuls run in fp8(E4M3) with perf_mode=DoubleRow (two contraction
rows per PE cell): operands live in 3D SBUF tiles [128, ksub, free] and
each matmul consumes a [:, k:k+2, :] slice. Numerics:
  - host scales W by 32 before the fp8 cast (keeps N(0,1/1024) weights
    out of the fp8 subnormal range); the q/k factors cancel inside
    exp's scale (2^-15) and V's factor is folded into 1/colsum.
  - exp is shifted by -3 so attn values stay below fp8e4's +-240 max
    (softmax is shift-invariant).
  - scores/colsum/outT accumulate in fp32 PSUM; the normalization,
    output mask (folded into Wv on host) and fp32 residual are applied
    in the final fp32 stage.

The SPMD program is identical on all cores; the AllGather delivers keys
in global order for everyone.
"""

import numpy as np
import ml_dtypes

import concourse.bass as bass
import concourse.mybir as mybir
import concourse.tile as tile
from concourse import bacc, bass_utils

B, S, D = 4, 2048, 1024
N_CORES = 8
SH = S // 2          # queries / own keys per core
PD = 128             # partition dim
KD = D // PD         # 8 ksub chunks over d
KH = SH // PD        # 8 ksub chunks over own keys
KC = S // PD         # 16 ksub chunks over all keys
NT = 512             # matmul free-dim tile (one PSUM bank of fp32)
F8 = mybir.dt.float8e4
F32 = mybir.dt.float32
W_SCALE = 32.0
EXP_SCALE = 1.0 / (32.0 * W_SCALE * W_SCALE)   # 1/sqrt(D) / W_SCALE^2
EXP_SHIFT = -3.0
DR = mybir.MatmulPerfMode.DoubleRow

_FP8 = ml_dtypes.float8_e4m3
_GROUPS = [[0, 1], [2, 3], [4, 5], [6, 7]]


def _build_kernel(tc):
    nc = tc.nc
    seqTq = nc.dram_tensor("seqTq", [D, SH], F8, kind="ExternalInput").ap()
    wq = nc.dram_tensor("wq", [D, D], F8, kind="ExternalInput").ap()
    wk = nc.dram_tensor("wk", [D, D], F8, kind="ExternalInput").ap()
    wv = nc.dram_tensor("wv", [D, D], F8, kind="ExternalInput").ap()
    seqTh = nc.dram_tensor("seqTh", [SH, D], F32, kind="ExternalInput").ap()
    outT = nc.dram_tensor("outT", [SH, D], F32, kind="ExternalOutput").ap()

    Exp = mybir.ActivationFunctionType.Exp

    with (
        tc.tile_pool(name="p_seq", bufs=1) as p_seq,
        tc.tile_pool(name="p_w", bufs=2) as p_w,
        tc.tile_pool(name="p_own", bufs=1) as p_own,
        tc.tile_pool(name="p_qt", bufs=1) as p_qt,
        tc.tile_pool(name="p_kt", bufs=1) as p_kt,
        tc.tile_pool(name="p_v", bufs=1) as p_v,
        tc.tile_pool(name="p_at", bufs=1) as p_at,
        tc.tile_pool(name="p_sh", bufs=4) as p_sh,
        tc.tile_pool(name="p_o", bufs=4) as p_o,
        tc.tile_pool(name="p_msc", bufs=1) as p_msc,
        tc.tile_pool(name="p_dram", bufs=1, space="DRAM") as p_dram,
        tc.tile_pool(name="p_mm", bufs=6, space="PSUM") as p_mm,
        tc.tile_pool(name="p_cs", bufs=1, space="PSUM") as p_cs,
    ):
        # ---- resident inputs ------------------------------------------------
        def load_w(w_dram, label):
            t = p_w.tile([PD, KD, D], F8, tag="w", name=label)
            for j in range(KD):
                nc.sync.dma_start(t[:, j, :], w_dram[j * PD:(j + 1) * PD, :])
            return t

        # Wk first (first matmul chain needs it), then own-half seq columns.
        wk3 = load_w(wk, "wk3")
        seqq3 = p_seq.tile([PD, KD, SH], F8, tag="seqq", name="seqq3")
        for j in range(KD):
            nc.sync.dma_start(seqq3[:, j, :], seqTq[j * PD:(j + 1) * PD, :])

        # collective bounce buffers (DRAM, Local)
        ib_kt = p_dram.tile([D, SH], F8, tag="ibk", name="ib_kt")
        ob_kt = p_dram.tile([2, D, SH], F8, tag="obk", name="ob_kt")
        ib_v = p_dram.tile([SH, D], F8, tag="ibv", name="ib_v")
        ob_v = p_dram.tile([2, SH, D], F8, tag="obv", name="ob_v")

        # ---- KT_own = (seq_own @ Wk).T, bounce out (ACT queue), AllGather ---
        kto3 = p_own.tile([PD, KD, SH], F8, tag="kto", name="kto3")
        for m in range(KD):
            for n in range(SH // NT):
                ps = p_mm.tile([PD, NT], F32, tag="mm", name=f"ps_k{m}_{n}")
                for k in range(0, KD, 2):
                    nc.tensor.matmul(
                        ps[:],
                        wk3[:, k:k + 2, m * PD:(m + 1) * PD],
                        seqq3[:, k:k + 2, n * NT:(n + 1) * NT],
                        start=(k == 0),
                        stop=(k == KD - 2),
                        perf_mode=DR,
                    )
                nc.vector.tensor_copy(kto3[:, m, n * NT:(n + 1) * NT], ps[:])
            nc.scalar.dma_start(ib_kt[m * PD:(m + 1) * PD, :], kto3[:, m, :])
        nc.gpsimd.collective_compute(
            "AllGather", mybir.AluOpType.bypass, replica_groups=_GROUPS,
            ins=[ib_kt.opt()], outs=[ob_kt.opt()],
        )

        # ---- V_own = seq_own @ (Wv * mask), bounce out, AllGather -----------
        wv3 = load_w(wv, "wv3")
        vo3 = p_own.tile([PD, KH, D], F8, tag="vo", name="vo3")
        for m in range(KH):
            for n in range(D // NT):
                ps = p_mm.tile([PD, NT], F32, tag="mm", name=f"ps_v{m}_{n}")
                for k in range(0, KD, 2):
                    nc.tensor.matmul(
                        ps[:],
                        seqq3[:, k:k + 2, m * PD:(m + 1) * PD],
                        wv3[:, k:k + 2, n * NT:(n + 1) * NT],
                        start=(k == 0),
                        stop=(k == KD - 2),
                        perf_mode=DR,
                    )
                nc.vector.tensor_copy(vo3[:, m, n * NT:(n + 1) * NT], ps[:])
            nc.scalar.dma_start(ib_v[m * PD:(m + 1) * PD, :], vo3[:, m, :])
        nc.gpsimd.collective_compute(
            "AllGather", mybir.AluOpType.bypass, replica_groups=_GROUPS,
            ins=[ib_v.opt()], outs=[ob_v.opt()],
        )

        # ---- QT = (seq_own @ Wq).T (overlaps the collectives) ---------------
        wq3 = load_w(wq, "wq3")
        qt3 = p_qt.tile([PD, KD, SH], F8, tag="qt", name="qt3")
        for m in range(KD):
            for n in range(SH // NT):
                ps = p_mm.tile([PD, NT], F32, tag="mm", name=f"ps_q{m}_{n}")
                for k in range(0, KD, 2):
                    nc.tensor.matmul(
                        ps[:],
                        wq3[:, k:k + 2, m * PD:(m + 1) * PD],
                        seqq3[:, k:k + 2, n * NT:(n + 1) * NT],
                        start=(k == 0),
                        stop=(k == KD - 2),
                        perf_mode=DR,
                    )
                nc.vector.tensor_copy(qt3[:, m, n * NT:(n + 1) * NT], ps[:])

        # ---- gather exchanged KT / V into SBUF (GpSimd queue: idle, and the
        # Sync queue's bulk input loads must not delay these) ----------------
        kt3 = p_kt.tile([PD, KD, S], F8, tag="kt", name="kt3")
        for m in range(KD):
            for r in range(2):
                nc.gpsimd.dma_start(
                    kt3[:, m, r * SH:(r + 1) * SH],
                    ob_kt[r, m * PD:(m + 1) * PD, :],
                )
        v3t = p_v.tile([PD, KC, D], F8, tag="v", name="v3t")
        for m in range(KC):
            r, mm_ = divmod(m, KH)
            nc.gpsimd.dma_start(v3t[:, m, :], ob_v[r, mm_ * PD:(mm_ + 1) * PD, :])

        # ---- scoresT -> exp(shifted) -> colsum ------------------------------
        ones3 = p_msc.tile([PD, 2, 16], F8, tag="ones", name="ones3")
        nc.vector.memset(ones3[:], 1.0)
        ebias = p_msc.tile([PD, 1], F32, tag="ebias", name="ebias")
        nc.vector.memset(ebias[:], EXP_SHIFT)
        cs_ps = p_cs.tile([1, SH], F32, tag="cs", name="cs")
        at3 = p_at.tile([PD, KC, SH], F8, tag="at", name="at3")

        def colsum_mm(m):
            for n in range(SH // NT):
                nc.tensor.matmul(
                    cs_ps[:, n * NT:(n + 1) * NT],
                    ones3[:, 0:2, 0:1],
                    at3[:, m:m + 2, n * NT:(n + 1) * NT],
                    start=(m == 0),
                    stop=(m == KC - 2),
                    perf_mode=DR,
                )

        for m in range(KC):
            for n in range(SH // NT):
                ps = p_mm.tile([PD, NT], F32, tag="mm", name=f"ps_s{m}_{n}")
                for k in range(0, KD, 2):
                    nc.tensor.matmul(
                        ps[:],
                        kt3[:, k:k + 2, m * PD:(m + 1) * PD],
                        qt3[:, k:k + 2, n * NT:(n + 1) * NT],
                        start=(k == 0),
                        stop=(k == KD - 2),
                        perf_mode=DR,
                    )
                nc.scalar.activation(
                    at3[:, m, n * NT:(n + 1) * NT], ps[:], Exp,
                    bias=ebias[:], scale=EXP_SCALE,
                )
            # colsum pairs ksubs (m, m+1); emit one pair late so the PE
            # never waits on ACT's exp
            if m >= 3 and m % 2 == 1:
                colsum_mm(m - 3)
        colsum_mm(KC - 2)

        # ---- 1/colsum (incl. V's W_SCALE), transposed to per-partition ------
        recip_sb = p_msc.tile([1, SH], F32, tag="recip", name="recip")
        nc.vector.reciprocal(recip_sb[:], cs_ps[:])
        nc.vector.tensor_scalar_mul(recip_sb[:], recip_sb[:], 1.0 / W_SCALE)
        recipT = p_msc.tile([PD, KH], F32, tag="recipT", name="recipT")
        for m in range(KH):
            nc.gpsimd.dma_start(recipT[:, m:m + 1], recip_sb[0:1, m * PD:(m + 1) * PD])

        # ---- O = AT.T @ V in [q, d] layout; fused normalize + residual ------
        # out[q, d] = (sum_key at[key, q] * v[key, d]) * recip[q] + seq[q, d]
        for m in range(KH):
            sh_t = p_sh.tile([PD, D], F32, tag="sh", name=f"sh{m}")
            nc.sync.dma_start(sh_t[:], seqTh[m * PD:(m + 1) * PD, :])
            o_t = p_o.tile([PD, D], F32, tag="o", name=f"o{m}")
            for n in range(D // NT):
                ps = p_mm.tile([PD, NT], F32, tag="mm", name=f"ps_o{m}_{n}")
                for k in range(0, KC, 2):
                    nc.tensor.matmul(
                        ps[:],
                        at3[:, k:k + 2, m * PD:(m + 1) * PD],
                        v3t[:, k:k + 2, n * NT:(n + 1) * NT],
                        start=(k == 0),
                        stop=(k == KC - 2),
                        perf_mode=DR,
                    )
                nc.vector.scalar_tensor_tensor(
                    o_t[:, n * NT:(n + 1) * NT],
                    ps[:],
                    recipT[:, m:m + 1],
                    sh_t[:, n * NT:(n + 1) * NT],
                    op0=mybir.AluOpType.mult,
                    op1=mybir.AluOpType.add,
                )
            nc.sync.dma_start(outT[m * PD:(m + 1) * PD, :], o_t[:])


_NC_CACHE = None


def _get_nc():
    global _NC_CACHE
    if _NC_CACHE is None:
        nc = bacc.Bacc(
            "TRN2", target_bir_lowering=False, debug=False, num_devices=N_CORES
        )
        with tile.TileContext(nc) as tc:
            _build_kernel(tc)
        nc.compile()
        _NC_CACHE = nc
    return _NC_CACHE


def _prep_in_maps(seq, Wq, Wk, Wv, mask):
    seq = np.asarray(seq, dtype=np.float32)
    wq_f8 = (np.asarray(Wq, dtype=np.float32) * W_SCALE).astype(_FP8)
    wk_f8 = (np.asarray(Wk, dtype=np.float32) * W_SCALE).astype(_FP8)
    wvm_f8 = (np.asarray(Wv, dtype=np.float32)
              * np.asarray(mask, dtype=np.float32)[None, :] * W_SCALE).astype(_FP8)
    in_maps = []
    for c in range(N_CORES):
        b, h = divmod(c, 2)
        seqT_own = np.ascontiguousarray(seq[b, h * SH:(h + 1) * SH, :].T)  # [D, SH]
        in_maps.append({
            "seqTq": seqT_own.astype(_FP8),
            "wq": wq_f8,
            "wk": wk_f8,
            "wv": wvm_f8,
            "seqTh": np.ascontiguousarray(seq[b, h * SH:(h + 1) * SH, :]),
        })
    return in_maps


def _run(seq, Wq, Wk, Wv, mask, trace=False, **run_kwargs):
    nc = _get_nc()
    in_maps = _prep_in_maps(seq, Wq, Wk, Wv, mask)
    res = bass_utils.run_bass_kernel_spmd(
        nc, in_maps, core_ids=list(range(N_CORES)), trace=trace, **run_kwargs
    )
    out = np.empty((B, S, D), dtype=np.float32)
    for c in range(N_CORES):
        b, h = divmod(c, 2)
        out[b, h * SH:(h + 1) * SH, :] = res.results[c]["outT"]
    return out, res


def kernel(seq, Wq, Wk, Wv, mask):
    out, _ = _run(seq, Wq, Wk, Wv, mask)
    return out
